# revision 34
# baseline (speedup 1.0000x reference)
"""Sigmoid-attention block kernel for trn2 (one NeuronCore, SPMD over 8).

fp8-DoubleRow attention with decoupled PSUM rings (~109us baseline -> v2
prologue/tail overhaul). Host prep: qT2 [128, 2(ic), 2(hpb), 512] bf16
(= query.T slab, ic-major pieces), WqT2 [128, 2(hpb), H] bf16 (= Wq.T);
key8c/val8c [8, 128, 2, CW] e4m3 (chunk-major DoubleRow interleave:
plane o = contraction rows 128o..128o+127); WkT8/WvT8 [128, 2, H] e4m3;
bq2/bk2 [128, 2] fp32 (hb-major), bv2 [128, 512] fp32 (bv twice).

Per core:
  qT   = Wq-blocks @ queryT + bq      (bf16)     -> e4m3 [128, 2, SLAB]
  kT   = Wk .T2 @ keyT8 + bk          (fp8 DR)   -> e4m3 [128, 2, N]
  vp   = valueT8 .T2 @ WvT8 + bv      (fp8 DR)   -> e4m3 [128, 2, H]/pair
  attnT[j] = sigmoid(kT .T2 @ qT)     (fp8 DR; ACT sigmoid [128,1024]
             psum -> e4m3; all 32 pair-tiles retained in SBUF)
  outT[0:128]   += vp .T2 @ attnT     (fp8 DR, inline over 32 pairs)
  outT[128:256] += vp .T2 @ attnT     (fp8 DR, phase B re-reading the
             retained attnT tiles)

Changes vs the 110us baseline (same math, same numerics, ~104us):
  - head DMAs split across the three DMA-issue queues (sync HW, scalar
    HW, gpsimd swDGE) instead of all-serial on sync. Each DMA costs
    ~1.5-2us of descriptor-pipeline latency plus transfer at
    ~150-250GB/s per queue (2KB per-partition packets), so tensors are
    host-packed into few fully-contiguous transfers, ordered by
    first-use: sync [kc0, qf-ic0, qf-ic1, kc1], scalar [wq2, wk8],
    gpsimd [bqk, vc0, wv8, bv2, vc1].
  - queryT DMA'd in two ic-pieces; qT projected/cast per (hb, ic)
    (skip_group_check - the ic psum regions are disjoint) so the first
    logits matmuls start before the full slab has even arrived; the
    first two pairs' sigmoids fire per ic-half ([512] ACTIVATEs).
    qT-hb0 psum in psO, qT-hb1 in a psL slot (fully parallel on PE).
  - warmup stationary = DVE-memset zeros (drops the gpsimd
    make_identity dependency from the PE ramp path).
  - steady-state kv chunk DMAs two chunks ahead; vc on the gpsimd
    queue (kc on sync), keeping the scalar queue sigmoid-only.
  - phase-B matmuls wait-paced (tile_wait_until) to ~2 per pair-period:
    the scheduler's ACT cost model runs ~10% slow vs hardware, so left
    alone it over-packs phase B into the last chunks' modeled PE slack
    and starves the real sigmoid stream for ~7us.
  - drain: half-copies on DVE/ACT, 4 output DMAs alternating the two
    HW queues (the swDGE queue's latency would gate the final DMA).

PSUM rings (8 banks): psL 2x[128,1024] carries ONLY logits psums (the
ACT sigmoid stream is the pacemaker); psP 2x[128,512] carries kT/vproj
psums and becomes the h-block-1 accumulator in phase B; psO 1x[128,1024]
is the inline h-block-0 accumulator (hosts the qT-hb0 psum in the
prologue, while qT-hb1 borrows a psL slot).
"""
from contextlib import ExitStack

import concourse.bass as bass
import concourse.mybir as mybir
import concourse.tile as tile
from concourse import bacc

F32 = mybir.dt.float32
BF16 = mybir.dt.bfloat16
E4 = mybir.dt.float8e4
AF = mybir.ActivationFunctionType
DR = mybir.MatmulPerfMode.DoubleRow


def _build_attn_kernel(SLAB=1024, N=8192, H=256):
    assert H == 256
    HB = H // 128            # 2 h-blocks
    NJ = N // 128            # 64 j-blocks
    NP = NJ // 2             # 32 j-pairs
    CW = 1024                # key/value chunk width (j cols)
    NCH = N // CW            # 8 chunks
    PPC = CW // 256          # 4 pairs per chunk
    IC = SLAB // 512         # 2 i sub-blocks

    nc = bacc.Bacc()
    qT2 = nc.dram_tensor("qT2", [IC, 128, HB, 512], BF16, kind="ExternalInput")
    key8c = nc.dram_tensor("key8c", [NCH, 128, 2, CW], E4, kind="ExternalInput")
    val8c = nc.dram_tensor("val8c", [NCH, 128, 2, CW], E4, kind="ExternalInput")
    WqT2 = nc.dram_tensor("WqT2", [128, HB, H], BF16, kind="ExternalInput")
    WkT8 = nc.dram_tensor("WkT8", [128, 2, H], E4, kind="ExternalInput")
    WvT8 = nc.dram_tensor("WvT8", [128, 2, H], E4, kind="ExternalInput")
    bqkd = nc.dram_tensor("bqk", [128, 4], F32, kind="ExternalInput")
    bv2d = nc.dram_tensor("bv2", [128, 512], F32, kind="ExternalInput")
    outd = nc.dram_tensor("outT", [H, SLAB], F32, kind="ExternalOutput")

    with tile.TileContext(nc) as tc, ExitStack() as ctx:
        cpool = ctx.enter_context(tc.tile_pool(name="const", bufs=1))
        psO = ctx.enter_context(tc.tile_pool(name="psO", bufs=1, space="PSUM"))
        psL = ctx.enter_context(tc.tile_pool(name="psL", bufs=2, space="PSUM"))
        psP = ctx.enter_context(tc.tile_pool(name="psP", bufs=2, space="PSUM"))
        kqp = ctx.enter_context(tc.tile_pool(name="kqp", bufs=1))
        atp = ctx.enter_context(tc.tile_pool(name="atp", bufs=NP + 2))
        vpp = ctx.enter_context(tc.tile_pool(name="vpp", bufs=NP + 2))
        kcp = ctx.enter_context(tc.tile_pool(name="kcp", bufs=3))
        vcp = ctx.enter_context(tc.tile_pool(name="vcp", bufs=3))
        outp = ctx.enter_context(tc.tile_pool(name="outp", bufs=1))

        # zeros for PE warmup + ACT table warm (DVE memset: no gpsimd dep)
        wz = cpool.tile([128, 128], F32, tag="wz")
        nc.vector.memset(wz[:], 0.0)

        # ---- head DMAs, split across the three issue queues; each DMA
        # has a fully-contiguous DRAM source and ~0.9-2us pipeline cost.
        # Queues are ordered by first-use time, and non-critical V-side
        # transfers are wait-paced off the contended prologue window so
        # the aggregate HBM draw stays with the qf/kc0/wq2 chain. ----
        # sync HW queue: K chunk 0 (gates the first logits via the kT
        # quarters), then the Q-side slab pieces, then K chunk 1
        kc0 = kcp.tile([128, 2, CW], E4, tag="kc", name="kc0")
        nc.sync.dma_start(kc0[:], key8c[0, :, :, :])
        qf = cpool.tile([128, IC, HB, 512], BF16, tag="qf")
        for ic in range(IC):
            nc.sync.dma_start(qf[:, ic, :, :], qT2[ic, :, :, :])
        kc1 = kcp.tile([128, 2, CW], E4, tag="kc", name="kc1")
        nc.sync.dma_start(kc1[:], key8c[1, :, :, :])
        # scalar HW queue: just the small q/k weights (frees ACT early)
        wq2 = cpool.tile([128, HB, H], BF16, tag="wq2")
        nc.scalar.dma_start(wq2[:], WqT2[:, :, :])
        wk8 = cpool.tile([128, 2, H], E4, tag="wk8")
        nc.scalar.dma_start(wk8[:], WkT8[:, :, :])
        # gpsimd swDGE queue: packed biases first, then V-side
        bqk = cpool.tile([128, 4], F32, tag="bqk")
        nc.gpsimd.dma_start(bqk[:], bqkd[:, :])
        vc0 = vcp.tile([128, 2, CW], E4, tag="vc", name="vc0")
        nc.gpsimd.dma_start(vc0[:], val8c[0, :, :, :])
        wv8 = cpool.tile([128, 2, H], E4, tag="wv8")
        nc.gpsimd.dma_start(wv8[:], WvT8[:, :, :])
        bv2_t = cpool.tile([128, 512], F32, tag="bv2")
        nc.gpsimd.dma_start(bv2_t[:], bv2d[:, :])
        vc1 = vcp.tile([128, 2, CW], E4, tag="vc", name="vc1")
        nc.gpsimd.dma_start(vc1[:], val8c[1, :, :, :])

        # sigmoid table preload (after the scalar-queue DMA issues)
        sgw = cpool.tile([128, 1], F32, tag="sgw")
        nc.scalar.activation(sgw[:], wz[:, :1], AF.Sigmoid)

        # short HAM warmup bridging the gap until the first DMA lands
        pwarm = psL.tile([128, 1024], F32, tag="ps", name="pwarm")
        for r in range(5):
            nc.tensor.matmul(pwarm[:, :128], wz[:], wz[:],
                             start=True, stop=True)

        # ---- qT projection -> e4m3: hb0 psum in psO, hb1 in a psL slot;
        # per-(hb, ic) casts so logits can start on the ic0 half early.
        # hb0 casts on ACT (idle pre-stream), hb1 casts on DVE.
        qT_f8 = kqp.tile([128, 2, SLAB], E4, tag="qT_f8")
        pq0 = psO.tile([128, SLAB], F32, tag="po", name="pq0")
        pq1 = psL.tile([128, SLAB], F32, tag="ps", name="pq1")
        pq = [pq0, pq1]
        for ic in range(IC):
            for hb in range(HB):
                for hpb in range(HB):
                    # disjoint ic regions: skip the accumulation-group
                    # check so ic1 matmuls don't serialize on ic0 casts
                    nc.tensor.matmul(
                        pq[hb][:, ic * 512:(ic + 1) * 512],
                        wq2[:, hpb, hb * 128:(hb + 1) * 128],
                        qf[:, ic, hpb, :],
                        start=(hpb == 0), stop=(hpb == HB - 1),
                        skip_group_check=True,
                    )
            nc.scalar.add(
                qT_f8[:, 0, ic * 512:(ic + 1) * 512],
                pq0[:, ic * 512:(ic + 1) * 512], bqk[:, 0:1],
            )
            nc.vector.tensor_scalar_add(
                qT_f8[:, 1, ic * 512:(ic + 1) * 512],
                pq1[:, ic * 512:(ic + 1) * 512], bqk[:, 1:2],
            )

        kT_f8 = kqp.tile([128, 2, N], E4, tag="kT_f8")
        vp_tiles = [None] * NP
        at_tiles = [None] * NP

        def emit_kv_dma(c):
            kc = kcp.tile([128, 2, CW], E4, tag="kc", name="kc")
            nc.sync.dma_start(kc[:], key8c[c, :, :, :])
            vc = vcp.tile([128, 2, CW], E4, tag="vc", name="vc")
            nc.gpsimd.dma_start(vc[:], val8c[c, :, :, :])
            return kc, vc

        def emit_kT_quarter(c, kc, q):
            """Project (h-block, s-half) q of key chunk c: one psum grab."""
            hb, s = q // 2, q % 2
            pk = psP.tile([128, 512], F32, tag="pp", name="pk")
            nc.tensor.matmul(
                pk[:, :],
                wk8[:, :, hb * 128:(hb + 1) * 128],
                kc[:, :, s * 512:(s + 1) * 512],
                start=True, stop=True, perf_mode=DR,
            )
            lo = c * CW + s * 512
            nc.vector.tensor_scalar_add(
                kT_f8[:, hb, lo:lo + 512], pk[:, :], bqk[:, 2 + hb:3 + hb]
            )

        def emit_v_pair(c, vc, t):
            """Project j-pair t of value chunk c: one psum grab."""
            pv = psP.tile([128, 512], F32, tag="pp", name="pv")
            for o in range(2):
                jl = 2 * t + o
                nc.tensor.matmul(
                    pv[:, o * 256:(o + 1) * 256],
                    vc[:, :, jl * 128:(jl + 1) * 128],
                    wv8[:, :, :],
                    start=True, stop=True, perf_mode=DR,
                )
            p = c * PPC + t
            vp = vpp.tile([128, 2, H], E4, tag="vp", name="vp")
            nc.vector.tensor_add(vp[:, :, :], pv[:, :], bv2_t[:, :])
            vp_tiles[p] = vp

        def emit_pair_logits(p, split=False):
            at = atp.tile([128, 2, SLAB], E4, tag="at", name="at")
            for o in range(2):
                j = 2 * p + o
                pl = psL.tile([128, 1024], F32, tag="ps", name="pl")
                for ic in range(IC):
                    nc.tensor.matmul(
                        pl[:, ic * 512:(ic + 1) * 512],
                        kT_f8[:, :, j * 128:(j + 1) * 128],
                        qT_f8[:, :, ic * 512:(ic + 1) * 512],
                        start=True, stop=True, perf_mode=DR,
                        skip_group_check=split,
                    )
                    if split:
                        # prologue-only: fire the sigmoid per ic-half so
                        # the stream starts before the full slab of qT
                        # has even arrived
                        nc.scalar.activation(
                            at[:, o, ic * 512:(ic + 1) * 512],
                            pl[:, ic * 512:(ic + 1) * 512], AF.Sigmoid,
                        )
                if not split:
                    nc.scalar.activation(at[:, o, :], pl[:, :], AF.Sigmoid)
            at_tiles[p] = at

        def emit_out_acc_hb0(p):
            at, vp = at_tiles[p], vp_tiles[p]
            for ic in range(IC):
                nc.tensor.matmul(
                    po[:, ic * 512:(ic + 1) * 512],
                    vp[:, :, 0:128],
                    at[:, :, ic * 512:(ic + 1) * 512],
                    start=(p == 0), stop=(p == NP - 1), perf_mode=DR,
                )

        # ---- prologue: chunk 0 projections (DMAs already in flight);
        # shortest chain to the first sigmoid: both s0 kT quarters first,
        # then the first v-pairs; s1 lands during pairs 0-1's sigmoids
        emit_kT_quarter(0, kc0, 0)
        emit_kT_quarter(0, kc0, 2)
        emit_v_pair(0, vc0, 0)
        emit_v_pair(0, vc0, 1)
        emit_kT_quarter(0, kc0, 1)
        emit_kT_quarter(0, kc0, 3)
        emit_v_pair(0, vc0, 2)
        emit_v_pair(0, vc0, 3)

        # the inline h-block-0 accumulator reuses the qT-hb0 psum slot
        po = psO.tile([128, SLAB], F32, tag="po", name="po")

        # ---- main loop: psL carries only logits (sigmoid-paced ring);
        # one kT-quarter and one v-pair of chunk c+1 slot in per pair.
        # kv DMAs run two chunks ahead so the projections (and with them
        # the psP ring hand-off to phase B) can hoist early. ----
        chunk_tiles = {1: (kc1, vc1)}
        for c in range(NCH):
            if c + 2 < NCH:
                chunk_tiles[c + 2] = emit_kv_dma(c + 2)
            nxt = chunk_tiles.get(c + 1)
            for t in range(PPC):
                p = c * PPC + t
                emit_pair_logits(p, split=(p < 2))
                if p >= 1:
                    emit_out_acc_hb0(p - 1)
                if nxt is not None:
                    kc, vc = nxt
                    emit_kT_quarter(c + 1, kc, t)
                    emit_v_pair(c + 1, vc, t)
        emit_out_acc_hb0(NP - 1)

        # ---- phase B: h-block-1 accumulation in the freed psP slots,
        # re-reading the retained attnT/vp tiles. The scheduler's ACT
        # cost model runs slow vs hardware, so left alone it over-packs
        # these into the last chunks' PE and starves the sigmoid stream;
        # wait-pacing spreads them at ~2 matmuls per pair-period. ----
        po1 = [psP.tile([128, 512], F32, tag="pp", name=f"po1_{ic}")
               for ic in range(IC)]
        for p in range(NP):
            at, vp = at_tiles[p], vp_tiles[p]
            with tc.tile_wait_until(0.050 + 0.00145 * p, enable=(p < 30)):
                for ic in range(IC):
                    nc.tensor.matmul(
                        po1[ic][:, :],
                        vp[:, :, 128:256],
                        at[:, :, ic * 512:(ic + 1) * 512],
                        start=(p == 0), stop=(p == NP - 1), perf_mode=DR,
                    )

        # ---- drain: half-copies spread over DVE/GpSimd/ACT, output
        # DMAs spread over the three issue queues ----
        ot = [outp.tile([128, SLAB], F32, tag=f"ot{hb}", name=f"ot{hb}")
              for hb in range(HB)]
        nc.vector.tensor_copy(ot[0][:, 0:512], po[:, 0:512])
        nc.scalar.copy(ot[0][:, 512:1024], po[:, 512:1024])
        nc.vector.tensor_copy(ot[1][:, 0:512], po1[0][:, :])
        nc.scalar.copy(ot[1][:, 512:1024], po1[1][:, :])
        # both HW queues; the swDGE queue's ~2us latency would gate the
        # final DMA
        dmas = [nc.sync, nc.scalar, nc.sync, nc.scalar]
        for hb in range(HB):
            for s in range(2):
                dmas[hb * 2 + s].dma_start(
                    outd[hb * 128:(hb + 1) * 128, s * 512:(s + 1) * 512],
                    ot[hb][:, s * 512:(s + 1) * 512],
                )

    nc.finalize()
    return nc


import numpy as np
import ml_dtypes
from concourse.bass_utils import run_bass_kernel_spmd

BF = ml_dtypes.bfloat16
N_CORES = 8
N_FULL = 8192
H_FULL = 256
SLAB_FULL = N_FULL // N_CORES

_NC = None


def _get_nc():
    global _NC
    if _NC is None:
        _NC = _build_attn_kernel(SLAB=SLAB_FULL, N=N_FULL, H=H_FULL)
    return _NC


def _in_maps(inputs):
    import concourse.mybir as mybir
    E4NP = mybir.dt.np(mybir.dt.float8e4)
    full = {k: np.asarray(v, dtype=np.float32) for k, v in inputs.items()}
    N, H = full["key"].shape[0], full["key"].shape[1]
    NCH, CW = 8, 1024
    queryT = full["query"].T.astype(BF)                       # [H, N]
    # key/value in fp8 DoubleRow layout, chunk-major [NCH, 128, 2, CW]:
    # plane o holds contraction rows 128o..128o+127
    keyT8 = full["key"].T.reshape(2, 128, N).transpose(1, 0, 2)
    key8c = keyT8.reshape(128, 2, NCH, CW).transpose(2, 0, 1, 3)
    valueT8 = full["value"].T.reshape(2, 128, N).transpose(1, 0, 2)
    val8c = valueT8.reshape(128, 2, NCH, CW).transpose(2, 0, 1, 3)
    WkT8 = full["Wk"].T.reshape(2, 128, H).transpose(1, 0, 2)
    WvT8 = full["Wv"].T.reshape(2, 128, H).transpose(1, 0, 2)
    WqT2 = full["Wq"].T.reshape(2, 128, H).transpose(1, 0, 2)  # [p, hpb, h]
    bqk = np.stack([full["bq"].reshape(2, 128)[0],
                    full["bq"].reshape(2, 128)[1],
                    full["bk"].reshape(2, 128)[0],
                    full["bk"].reshape(2, 128)[1]], axis=1)   # [128, 4]
    shared = {
        "key8c": np.ascontiguousarray(key8c.astype(E4NP)),
        "val8c": np.ascontiguousarray(val8c.astype(E4NP)),
        "WqT2": np.ascontiguousarray(WqT2.astype(BF)),
        "WkT8": np.ascontiguousarray(WkT8.astype(E4NP)),
        "WvT8": np.ascontiguousarray(WvT8.astype(E4NP)),
        "bqk": np.ascontiguousarray(bqk),
        "bv2": np.ascontiguousarray(np.tile(full["bv"][None, :], (128, 2))),
    }
    maps = []
    for c in range(N_CORES):
        m = dict(shared)
        qslab = queryT[:, c * SLAB_FULL:(c + 1) * SLAB_FULL]  # [H, SLAB]
        # qT2[ic, p, hpb, j] = qslab[hpb*128+p, ic*512+j]
        qT2 = qslab.reshape(2, 128, 2, 512).transpose(2, 1, 0, 3)
        m["qT2"] = np.ascontiguousarray(qT2)
        maps.append(m)
    return maps


def kernel(**inputs) -> np.ndarray:
    nc = _get_nc()
    res = run_bass_kernel_spmd(nc, _in_maps(inputs), list(range(N_CORES)))
    return np.ascontiguousarray(np.concatenate(
        [np.asarray(res.results[c]["outT"]).T for c in range(N_CORES)],
        axis=0,
    )).astype(np.float32)
